# revision 4
# baseline (speedup 1.0000x reference)
"""Trainium2 Bass kernel for nn_CNNEmbedding: char-CNN word embedding.

Reference computation (per flattened word, NW=16384 words):
  x = emb[char_ids]                       # [16, 64]
  for w in 1..6: y_w = conv1d(x.T, W_w, 'wide' pad) ; f_w = max_t tanh(y_w + b_w)
  f = concat(f_w)                         # [525]
  out[word_pos, word_batch] = f           # [256, 64, 525]

Kernel strategy (8 NeuronCores, data-parallel over words, 2048 words/core):
  - tanh is monotonic => max-pool BEFORE bias+tanh.
  - embedding lookup via SWDGE dma_gather (transpose mode) from a bf16 table
    whose 256-byte rows are [emb[v] | emb[v]]; row 256 is zeros (padding).
    This lands x directly as [128 partitions (d duplicated), word-cols] in
    SBUF. Word blocks are strided 21 cols (16 chars + 5 shared zero pads).
  - partitions 64..127 are then shifted left by one column, so a single
    K=128 matmul computes TWO conv taps (dt, dt+1) at once.
  - each conv = ceil(w/2) shifted matmuls accumulating in PSUM (fp32);
    VectorE reduce_max straight out of PSUM; ScalarE fused bias+tanh;
    TensorE transposes [C, words] -> [words, C] for contiguous output DMA.
"""

import os
import numpy as np
import ml_dtypes

# ---- problem constants (hardcoded; kernel.py must be self-contained) ----
B = 64
WORDS = 256
NW = B * WORDS          # 16384
LMAX = 16
V = 256
D = 64
KS = [1, 2, 3, 4, 5, 6]
CS = [25, 50, 75, 100, 125, 150]
CTOT = sum(CS)          # 525

NCORES = 8
NWC = NW // NCORES      # 2048 words per core
GW = 512                # words per gather group
NGROUP = NWC // GW      # 4
S = 21                  # word stride in x-plane (16 chars + 5 shared zero pad)
DOFF = 5                # first char col within a word block
NGC = S * GW + DOFF     # 10757 meaningful cols per group
NIDXG = ((NGC + 127) // 128) * 128   # 10880 gather indices per group
IDXC = NIDXG // 16      # idx columns per group (680)
ZROW = 256              # zero row of the embedding table

# output column offset of each conv in the 525-wide feature vector
OUT_OFF = np.concatenate([[0], np.cumsum(CS)]).tolist()

_BF16 = ml_dtypes.bfloat16

_CACHE = {}


def _chains():
    """Conv chains: k6 is split into two 75-channel halves (PSUM partitions
    max 128). Returns list of dicts."""
    ch = []
    for ki, (w, c) in enumerate(zip(KS, CS)):
        if c <= 128:
            ch.append(dict(ki=ki, w=w, C=c, clo=0, out=OUT_OFF[ki], bias=None))
        else:
            h = c // 2
            ch.append(dict(ki=ki, w=w, C=h, clo=0, out=OUT_OFF[ki], bias=None))
            ch.append(dict(ki=ki, w=w, C=c - h, clo=h, out=OUT_OFF[ki] + h,
                           bias=None))
    for i, c in enumerate(ch):
        c["bias"] = i
    return ch


CHAINS = _chains()            # 7 chains
NCH = len(CHAINS)

# wall (weight) block layout: per conv k, ceil(w/2) blocks of C_k columns.
# Block = (col offset, K rows (128 pair / 64 odd), first tap dt).
WALL_BLOCKS = {}
_off = 0
for _ki, (_w, _c) in enumerate(zip(KS, CS)):
    blks = []
    for _p in range(_w // 2):
        blks.append((_off, 128, 2 * _p))
        _off += _c
    if _w % 2 == 1:
        blks.append((_off, 64, _w - 1))
        _off += _c
    WALL_BLOCKS[_ki] = blks
WALL_COLS = _off              # 1250


def _build_program():
    """Emit the Bass/Tile program (same program for all 8 cores)."""
    from contextlib import ExitStack

    import concourse.mybir as mybir
    import concourse.tile as tile
    from concourse import bacc
    from concourse.masks import make_identity

    dt = mybir.dt
    nc = bacc.Bacc("TRN2", target_bir_lowering=False, debug=False,
                   num_devices=NCORES)

    tab = nc.dram_tensor("tab", [V + 1, 128], dt.bfloat16,
                         kind="ExternalInput").ap()
    idx = nc.dram_tensor("idx", [128, NGROUP * IDXC], dt.int16,
                         kind="ExternalInput").ap()
    wall = nc.dram_tensor("wall", [128, WALL_COLS], dt.bfloat16,
                          kind="ExternalInput").ap()
    biasd = nc.dram_tensor("bias", [128, NCH], dt.float32,
                           kind="ExternalInput").ap()
    fout = nc.dram_tensor("f", [NWC, CTOT], dt.float32,
                          kind="ExternalOutput").ap()

    with tile.TileContext(nc) as tc, ExitStack() as ctx:
        singles = ctx.enter_context(tc.tile_pool(name="singles", bufs=1))
        xgp = ctx.enter_context(tc.tile_pool(name="xgp", bufs=2))
        xpp = ctx.enter_context(tc.tile_pool(name="xpp", bufs=2))
        psp = ctx.enter_context(tc.tile_pool(name="psp", bufs=2, space="PSUM"))
        trp = ctx.enter_context(tc.tile_pool(name="trp", bufs=2, space="PSUM"))
        fop = ctx.enter_context(tc.tile_pool(name="fop", bufs=2))

        idx_sb = singles.tile([128, NGROUP * IDXC], dt.int16, tag="idx")
        nc.sync.dma_start(out=idx_sb, in_=idx)
        wall_sb = singles.tile([128, WALL_COLS], dt.bfloat16, tag="wall")
        nc.sync.dma_start(out=wall_sb, in_=wall)
        bias_sb = singles.tile([128, NCH], dt.float32, tag="bias")
        nc.sync.dma_start(out=bias_sb, in_=biasd)
        ident = singles.tile([128, 128], dt.bfloat16, tag="ident")
        make_identity(nc, ident)

        feats = [
            singles.tile([ch["C"], NWC], dt.bfloat16, tag=f"feats{i}",
                         name=f"feats{i}")
            for i, ch in enumerate(CHAINS)
        ]

        # heavy chains first
        order = sorted(range(NCH), key=lambda i: -CHAINS[i]["w"])

        for g in range(NGROUP):
            xg = xgp.tile([128, NIDXG], dt.bfloat16, tag="xg")
            nc.gpsimd.dma_gather(
                out_ap=xg.rearrange("p (a n) -> p a n", a=1),
                in_ap=tab,
                idxs_ap=idx_sb[:, g * IDXC:(g + 1) * IDXC],
                num_idxs=NIDXG,
                num_idxs_reg=NIDXG,
                elem_size=128,
                transpose=True,
                single_packet=False,
            )
            # xp: top half = x, bottom half = x shifted left one column
            xp = xpp.tile([128, NIDXG], dt.bfloat16, tag="xp")
            nc.vector.tensor_copy(out=xp[0:64, :], in_=xg[0:64, :])
            nc.scalar.copy(out=xp[64:128, 0:NIDXG - 1], in_=xg[64:128, 1:NIDXG])

            for ci in order:
                ch = CHAINS[ci]
                w, C, clo = ch["w"], ch["C"], ch["clo"]
                T = LMAX + w - 1
                wpb = 512 // T                      # words per PSUM bank
                blocks = WALL_BLOCKS[ch["ki"]]
                # chunk list for this group
                chunks = []
                n0 = 0
                while n0 < GW:
                    chunks.append((n0, min(wpb, GW - n0)))
                    n0 += wpb
                # process in PSUM tiles of up to 3 banks
                for t0 in range(0, len(chunks), 3):
                    tg = chunks[t0:t0 + 3]
                    ps = psp.tile([C, 3, 512], dt.float32, tag="ps")
                    for j, (cn0, cnw) in enumerate(tg):
                        for bi, (boff, K, bdt) in enumerate(blocks):
                            o = DOFF + bdt - (w - 1)
                            rhs = (
                                xp[0:K, S * cn0 + o: S * cn0 + o + S * cnw]
                                .rearrange("p (n t) -> p n t", t=S)[:, :, 0:T]
                            )
                            nc.tensor.matmul(
                                ps[:, j, 0:cnw * T],
                                lhsT=wall_sb[0:K, boff + clo: boff + clo + C],
                                rhs=rhs,
                                start=(bi == 0),
                                stop=(bi == len(blocks) - 1),
                            )
                    # grouped max over time: one reduce per run of equal nw
                    r0 = 0
                    while r0 < len(tg):
                        r1 = r0
                        while r1 < len(tg) and tg[r1][1] == tg[r0][1]:
                            r1 += 1
                        na = r1 - r0
                        nwd = tg[r0][1]
                        src = ps[:, r0:r1, 0:nwd * T].rearrange(
                            "c a (n t) -> c a n t", t=T)
                        w0 = g * GW + tg[r0][0]
                        dst = feats[ci][:, w0: w0 + na * nwd].rearrange(
                            "c (a n) -> c a n", n=nwd)
                        nc.vector.reduce_max(out=dst, in_=src,
                                             axis=mybir.AxisListType.X)
                        r0 = r1

        # fused bias + tanh (in place, per chain)
        for i, ch in enumerate(CHAINS):
            nc.scalar.activation(
                out=feats[i], in_=feats[i],
                func=mybir.ActivationFunctionType.Tanh,
                bias=bias_sb[0:ch["C"], i:i + 1],
            )

        # transpose [C, words] -> [words, C] and DMA out.
        # bf16 PSUM writes need 4-byte alignment -> even column offsets in
        # the staging tile; the copy to fp32 restores the packed layout.
        even_off = []
        _eo = 0
        for ch in CHAINS:
            even_off.append(_eo)
            _eo += ch["C"] + (ch["C"] % 2)
        for wb in range(NWC // 128):
            tr = trp.tile([128, _eo], dt.bfloat16, tag="tr")
            for i, ch in enumerate(CHAINS):
                C = ch["C"]
                nc.tensor.transpose(
                    out=tr[:, even_off[i]: even_off[i] + C],
                    in_=feats[i][:, wb * 128:(wb + 1) * 128],
                    identity=ident[0:C, 0:C],
                )
            fo = fop.tile([128, CTOT], dt.float32, tag="fo")
            for i, ch in enumerate(CHAINS):
                C = ch["C"]
                nc.vector.tensor_copy(
                    out=fo[:, ch["out"]: ch["out"] + C],
                    in_=tr[:, even_off[i]: even_off[i] + C],
                )
            nc.sync.dma_start(out=fout[wb * 128:(wb + 1) * 128, :], in_=fo)

    nc.compile()
    return nc


def _host_consts(emb, Ws, bs):
    """Constant inputs shared by all cores: table, wall, bias."""
    tab = np.zeros((V + 1, 128), dtype=_BF16)
    e = emb.astype(_BF16)
    tab[:V, 0:64] = e
    tab[:V, 64:128] = e

    wall = np.zeros((128, WALL_COLS), dtype=_BF16)
    for ki, W in enumerate(Ws):
        Wb = W.astype(np.float32)
        for (boff, K, bdt) in WALL_BLOCKS[ki]:
            c = CS[ki]
            wall[0:64, boff:boff + c] = Wb[:, :, bdt].T.astype(_BF16)
            if K == 128:
                wall[64:128, boff:boff + c] = Wb[:, :, bdt + 1].T.astype(_BF16)

    bias = np.zeros((128, NCH), dtype=np.float32)
    for i, ch in enumerate(CHAINS):
        bsl = bs[ch["ki"]][ch["clo"]: ch["clo"] + ch["C"]]
        bias[0:ch["C"], i] = bsl
    return tab, wall, bias


def _host_idx(char_ids_core):
    """Per-core gather index plane: [128, NGROUP*IDXC] int16."""
    flat = np.full((NGROUP, NIDXG), ZROW, dtype=np.int16)
    n = np.arange(GW)
    t = np.arange(LMAX)
    pos = DOFF + S * n[:, None] + t[None, :]          # [512, 16]
    cid = char_ids_core.astype(np.int16).reshape(NGROUP, GW, LMAX)
    for g in range(NGROUP):
        flat[g, pos] = cid[g]
    flat = flat.reshape(NGROUP * NIDXG // 16, 16).T    # [16, ...]
    return np.tile(flat, (8, 1)).copy()                # [128, ...]


def kernel(**inputs):
    import jax

    jax.devices()  # boot the axon PJRT backend
    from concourse.bass_utils import run_bass_kernel_spmd

    char_ids = np.asarray(inputs["char_ids"], dtype=np.int32)
    word_pos = np.asarray(inputs["word_pos"], dtype=np.int64)
    word_batch = np.asarray(inputs["word_batch"], dtype=np.int64)
    emb = np.asarray(inputs["emb"], dtype=np.float32)
    Ws = [np.asarray(inputs[f"W{i+1}"], dtype=np.float32) for i in range(6)]
    bs = [np.asarray(inputs[f"b{i+1}"], dtype=np.float32) for i in range(6)]

    if "nc" not in _CACHE:
        _CACHE["nc"] = _build_program()
    nc = _CACHE["nc"]

    tab, wall, bias = _host_consts(emb, Ws, bs)
    in_maps = []
    for c in range(NCORES):
        in_maps.append({
            "tab": tab,
            "idx": _host_idx(char_ids[c * NWC:(c + 1) * NWC]),
            "wall": wall,
            "bias": bias,
        })

    core_ids = list(range(NCORES))
    trace = bool(os.environ.get("KERNEL_TRACE"))
    res = run_bass_kernel_spmd(nc, in_maps, core_ids, trace=trace)
    if trace:
        _CACHE["last_exec_time_ns"] = res.exec_time_ns

    f_full = np.concatenate([res.results[c]["f"] for c in core_ids], axis=0)

    out = np.zeros((WORDS, B, CTOT), dtype=np.float32)
    out[word_pos, word_batch] = f_full
    return out


# revision 11
# speedup vs baseline: 1.7707x; 1.7707x over previous
"""Trainium2 Bass kernel for nn_CNNEmbedding: char-CNN word embedding.

Reference computation (per flattened word, NW=16384 words):
  x = emb[char_ids]                       # [16, 64]
  for w in 1..6: y_w = conv1d(x.T, W_w, 'wide' pad) ; f_w = max_t tanh(y_w + b_w)
  f = concat(f_w)                         # [525]
  out[word_pos, word_batch] = f           # [256, 64, 525]

Kernel strategy (8 NeuronCores, data-parallel over words, 2048 words/core):
  - tanh is monotonic => max-pool BEFORE bias+tanh.
  - embedding lookup via one-hot matmul: ids broadcast to 128 partitions,
    VectorE tensor_scalar(is_equal) against a per-partition iota builds the
    one-hot [vocab-half, cols]; two accumulating matmuls against the
    (d-duplicated) embedding table produce x directly as
    [128 partitions (d dup), word-cols] in PSUM. Word blocks are strided
    21 cols (16 chars + 5 shared zero pad); pad slots use id 256 which
    matches no vocab row -> zero embedding.
  - ScalarE copies PSUM->SBUF: top half as-is, bottom half shifted left one
    column, so a single K=128 conv matmul computes TWO taps (dt, dt+1).
  - each conv = ceil(w/2) shifted matmuls accumulating in PSUM (fp32).
  - max over time: light chains via VectorE reduce_max from PSUM; heavy
    chains (k5, k6) via ScalarE PSUM->SBUF bf16 copy + GpSimd pairwise-max
    tree (engine load balancing).
  - ScalarE fused bias+tanh; TensorE transposes [C, words] -> [words, C]
    for contiguous output DMA.
"""

import os
import numpy as np
import ml_dtypes

# ---- problem constants (hardcoded; kernel.py must be self-contained) ----
B = 64
WORDS = 256
NW = B * WORDS          # 16384
LMAX = 16
V = 256
D = 64
KS = [1, 2, 3, 4, 5, 6]
CS = [25, 50, 75, 100, 125, 150]
CTOT = sum(CS)          # 525

NCORES = 8
NWC = NW // NCORES      # 2048 words per core
GW = 512                # words per group
NGROUP = NWC // GW      # 4
S = 21                  # word stride in x-plane (16 chars + 5 shared zero pad)
DOFF = 5                # first char col within a word block
NGC = S * GW + DOFF     # 10757 meaningful cols per group
NIDXG = ((NGC + 127) // 128) * 128   # 10880 cols per group plane
ECH = 512               # embed matmul chunk (one PSUM bank)
PAD_ID = 256            # pad id: matches no vocab row -> zero embedding

OUT_OFF = np.concatenate([[0], np.cumsum(CS)]).tolist()

_BF16 = ml_dtypes.bfloat16

_CACHE = {}


def _chains():
    ch = []
    for ki, (w, c) in enumerate(zip(KS, CS)):
        if c <= 128:
            ch.append(dict(ki=ki, w=w, C=c, clo=0, out=OUT_OFF[ki]))
        else:
            h = c // 2
            ch.append(dict(ki=ki, w=w, C=h, clo=0, out=OUT_OFF[ki]))
            ch.append(dict(ki=ki, w=w, C=c - h, clo=h, out=OUT_OFF[ki] + h))
    return ch


CHAINS = _chains()            # 7 chains
NCH = len(CHAINS)
# chains whose max-pool goes via ScalarE copy + GpSimd tree
# (disabled: trn2 walrus rejects max on the Pool engine)
OFFLOAD = set()

# wall (weight) block layout: per conv k, ceil(w/2) blocks of C_k columns.
WALL_BLOCKS = {}
_off = 0
for _ki, (_w, _c) in enumerate(zip(KS, CS)):
    blks = []
    for _p in range(_w // 2):
        blks.append((_off, 128, 2 * _p))
        _off += _c
    if _w % 2 == 1:
        blks.append((_off, 64, _w - 1))
        _off += _c
    WALL_BLOCKS[_ki] = blks
WALL_COLS = _off              # 1250


def _conv_items(mybir, nc, xp, wall_sb, psp, feats, scrp, g):
    """Yield emission callables for one group's conv work, per tile-group."""
    items = []
    for ci, ch in enumerate(CHAINS):
        w, C, clo = ch["w"], ch["C"], ch["clo"]
        T = LMAX + w - 1
        wpb = 512 // T
        blocks = WALL_BLOCKS[ch["ki"]]
        chunks = []
        n0 = 0
        while n0 < GW:
            chunks.append((n0, min(wpb, GW - n0)))
            n0 += wpb
        for t0 in range(0, len(chunks), 2):
            tg = chunks[t0:t0 + 2]
            items.append((ci, ch, T, blocks, tg))

    # round-robin across chains, heaviest chains first within each round
    by_chain = {}
    for it in items:
        by_chain.setdefault(it[0], []).append(it)
    order = sorted(by_chain, key=lambda c: -CHAINS[c]["w"])
    rr = []
    pos = 0
    while True:
        emitted = False
        for c in order:
            lst = by_chain[c]
            if pos < len(lst):
                rr.append(lst[pos])
                emitted = True
        if not emitted:
            break
        pos += 1
    return rr


def _build_program():
    from contextlib import ExitStack

    import concourse.mybir as mybir
    import concourse.tile as tile
    from concourse import bacc
    from concourse.masks import make_identity

    dt = mybir.dt
    nc = bacc.Bacc("TRN2", target_bir_lowering=False, debug=False,
                   num_devices=NCORES)

    idsd = nc.dram_tensor("ids", [1, NGROUP * NIDXG], dt.bfloat16,
                          kind="ExternalInput").ap()
    etab = nc.dram_tensor("etab", [128, 256], dt.bfloat16,
                          kind="ExternalInput").ap()
    iotad = nc.dram_tensor("iota", [128, 2], dt.float32,
                           kind="ExternalInput").ap()
    wall = nc.dram_tensor("wall", [128, WALL_COLS], dt.bfloat16,
                          kind="ExternalInput").ap()
    biasd = nc.dram_tensor("bias", [128, NCH], dt.float32,
                           kind="ExternalInput").ap()
    fout = nc.dram_tensor("f", [NWC, CTOT], dt.float32,
                          kind="ExternalOutput").ap()

    import concourse.bass as bass

    with tile.TileContext(nc) as tc, ExitStack() as ctx:
        singles = ctx.enter_context(tc.tile_pool(name="singles", bufs=1))
        idsp = ctx.enter_context(tc.tile_pool(name="idsp", bufs=2))
        ohp = ctx.enter_context(tc.tile_pool(name="ohp", bufs=4))
        xpp = ctx.enter_context(tc.tile_pool(name="xpp", bufs=2))
        psp = ctx.enter_context(tc.tile_pool(name="psp", bufs=2, space="PSUM"))
        psep = ctx.enter_context(tc.tile_pool(name="psep", bufs=1,
                                              space="PSUM"))
        trp = ctx.enter_context(tc.tile_pool(name="trp", bufs=2, space="PSUM"))
        scrp = ctx.enter_context(tc.tile_pool(name="scrp", bufs=4))
        fop = ctx.enter_context(tc.tile_pool(name="fop", bufs=2))

        etab_sb = singles.tile([128, 256], dt.bfloat16, tag="etab")
        nc.sync.dma_start(out=etab_sb, in_=etab)
        iota_sb = singles.tile([128, 2], dt.float32, tag="iota")
        nc.sync.dma_start(out=iota_sb, in_=iotad)
        wall_sb = singles.tile([128, WALL_COLS], dt.bfloat16, tag="wall")
        nc.sync.dma_start(out=wall_sb, in_=wall)
        bias_sb = singles.tile([128, NCH], dt.float32, tag="bias")
        nc.sync.dma_start(out=bias_sb, in_=biasd)
        ident = singles.tile([128, 128], dt.bfloat16, tag="ident")
        make_identity(nc, ident)

        feats = [
            singles.tile([ch["C"], NWC], dt.bfloat16, tag=f"feats{i}",
                         name=f"feats{i}")
            for i, ch in enumerate(CHAINS)
        ]

        def emit_embed_group(g):
            """Build xp plane for group g via one-hot matmuls. Returns a list
            of emission thunks (one per embed chunk-pair) so callers can weave
            them between conv items."""
            ids_sb = idsp.tile([128, NIDXG], dt.bfloat16, tag="ids",
                               name=f"ids{g}")
            bcast = bass.AP(
                tensor=idsd.tensor,
                offset=g * NIDXG,
                ap=[[0, 128], [1, NIDXG]],
            )
            nc.gpsimd.dma_start(out=ids_sb, in_=bcast)
            xp = xpp.tile([128, NIDXG], dt.bfloat16, tag="xp", name=f"xp{g}")

            thunks = []
            nch = (NIDXG + 2 * ECH - 1) // (2 * ECH)   # psum tiles of 2 chunks

            def mk(tci):
                def emit():
                    c0 = tci * 2 * ECH
                    cw = min(2 * ECH, NIDXG - c0)
                    pse = psep.tile([128, 2 * ECH], dt.float32, tag="pse",
                                    name=f"pse{g}_{tci}")
                    for j in range(0, cw, ECH):
                        n = min(ECH, cw - j)
                        for h in range(2):
                            oh = ohp.tile([128, ECH], dt.bfloat16, tag="oh",
                                          name=f"oh{g}_{tci}_{j}_{h}")
                            nc.vector.tensor_scalar(
                                out=oh[:, 0:n],
                                in0=ids_sb[:, c0 + j: c0 + j + n],
                                scalar1=iota_sb[:, h:h + 1],
                                scalar2=None,
                                op0=mybir.AluOpType.is_equal,
                            )
                            nc.tensor.matmul(
                                pse[:, j:j + n],
                                lhsT=etab_sb[:, h * 128:(h + 1) * 128],
                                rhs=oh[:, 0:n],
                                start=(h == 0),
                                stop=(h == 1),
                            )
                    # copy to xp: top as-is, bottom shifted left one column
                    nc.scalar.copy(out=xp[0:64, c0:c0 + cw],
                                   in_=pse[0:64, 0:cw])
                    if c0 == 0:
                        nc.scalar.copy(out=xp[64:128, 0:cw - 1],
                                       in_=pse[64:128, 1:cw])
                    else:
                        nc.scalar.copy(out=xp[64:128, c0 - 1:c0 + cw - 1],
                                       in_=pse[64:128, 0:cw])
                return emit

            for tci in range(nch):
                thunks.append(mk(tci))
            return xp, thunks

        def emit_conv_item(xp, g, item):
            ci, ch, T, blocks, tg = item
            C, clo, w = ch["C"], ch["clo"], ch["w"]
            ps = psp.tile([C, 2, 512], dt.float32, tag="ps",
                          name=f"ps{g}_{ci}_{tg[0][0]}")
            for j, (cn0, cnw) in enumerate(tg):
                for bi, (boff, K, bdt) in enumerate(blocks):
                    o = DOFF + bdt - (w - 1)
                    rhs = (
                        xp[0:K, S * cn0 + o: S * cn0 + o + S * cnw]
                        .rearrange("p (n t) -> p n t", t=S)[:, :, 0:T]
                    )
                    nc.tensor.matmul(
                        ps[:, j, 0:cnw * T],
                        lhsT=wall_sb[0:K, boff + clo: boff + clo + C],
                        rhs=rhs,
                        start=(bi == 0),
                        stop=(bi == len(blocks) - 1),
                    )
            if ci not in OFFLOAD:
                # VectorE grouped reduce straight out of PSUM
                r0 = 0
                while r0 < len(tg):
                    r1 = r0
                    while r1 < len(tg) and tg[r1][1] == tg[r0][1]:
                        r1 += 1
                    na, nwd = r1 - r0, tg[r0][1]
                    src = ps[:, r0:r1, 0:nwd * T].rearrange(
                        "c a (n t) -> c a n t", t=T)
                    w0 = g * GW + tg[r0][0]
                    dst = feats[ci][:, w0: w0 + na * nwd].rearrange(
                        "c (a n) -> c a n", n=nwd)
                    nc.vector.reduce_max(out=dst, in_=src,
                                         axis=mybir.AxisListType.X)
                    r0 = r1
            else:
                # ScalarE copy to SBUF bf16, then GpSimd pairwise-max tree
                na = len(tg)
                sca = scrp.tile([C, 2, 512], dt.bfloat16, tag="scra",
                                name=f"scra{g}_{ci}_{tg[0][0]}")
                scb = scrp.tile([C, 2, 512], dt.bfloat16, tag="scrb",
                                name=f"scrb{g}_{ci}_{tg[0][0]}")
                rc = 0
                while rc < len(tg):
                    rc1 = rc
                    while rc1 < len(tg) and tg[rc1][1] == tg[rc][1]:
                        rc1 += 1
                    nwd = tg[rc][1]
                    nc.scalar.copy(out=sca[:, rc:rc1, 0:nwd * T],
                                   in_=ps[:, rc:rc1, 0:nwd * T])
                    rc = rc1
                r0 = 0
                while r0 < len(tg):
                    r1 = r0
                    while r1 < len(tg) and tg[r1][1] == tg[r0][1]:
                        r1 += 1
                    nrun, nwd = r1 - r0, tg[r0][1]
                    w0 = g * GW + tg[r0][0]

                    def view(t4, Lc):
                        return t4[:, r0:r1, 0:nwd * T].rearrange(
                            "c a (n t) -> c a n t", t=T)[:, :, :, 0:Lc]

                    L = T
                    cur, nxt = sca, scb
                    while (L + 1) // 2 > 1:
                        Lh = (L + 1) // 2
                        a = view(cur, L)
                        nc.gpsimd.tensor_tensor(
                            out=view(nxt, Lh),
                            in0=a[:, :, :, 0:Lh],
                            in1=a[:, :, :, L - Lh:L],
                            op=mybir.AluOpType.max,
                        )
                        cur, nxt = nxt, cur
                        L = Lh
                    # final level (L == 2) writes feats directly
                    a = view(cur, L)
                    dstf = feats[ci][:, w0: w0 + nrun * nwd].rearrange(
                        "c (a n) -> c a n", n=nwd)
                    nc.gpsimd.tensor_tensor(
                        out=dstf,
                        in0=a[:, :, :, 0],
                        in1=a[:, :, :, L - 1],
                        op=mybir.AluOpType.max,
                    )
                    r0 = r1

        # prologue: embed group 0
        xp_cur, thunks = emit_embed_group(0)
        for t in thunks:
            t()

        for g in range(NGROUP):
            items = _conv_items(mybir, nc, xp_cur, wall_sb, psp, feats, scrp,
                                g)
            if g + 1 < NGROUP:
                xp_next, nthunks = emit_embed_group(g + 1)
            else:
                xp_next, nthunks = None, []
            # weave: distribute embed thunks of next group between conv items
            ne, ni = len(nthunks), len(items)
            ti = 0
            for k, item in enumerate(items):
                emit_conv_item(xp_cur, g, item)
                want = (k + 1) * ne // ni
                while ti < want:
                    nthunks[ti]()
                    ti += 1
            while ti < ne:
                nthunks[ti]()
                ti += 1
            xp_cur = xp_next

        # fused bias + tanh (in place, per chain)
        for i, ch in enumerate(CHAINS):
            nc.scalar.activation(
                out=feats[i], in_=feats[i],
                func=mybir.ActivationFunctionType.Tanh,
                bias=bias_sb[0:ch["C"], i:i + 1],
            )

        # transpose [C, words] -> [words, C] and DMA out. bf16 PSUM writes
        # need 4-byte alignment -> even column offsets in the staging tile.
        even_off = []
        _eo = 0
        for ch in CHAINS:
            even_off.append(_eo)
            _eo += ch["C"] + (ch["C"] % 2)
        for wb in range(NWC // 128):
            tr = trp.tile([128, _eo], dt.bfloat16, tag="tr",
                          name=f"tr{wb}")
            for i, ch in enumerate(CHAINS):
                C = ch["C"]
                nc.tensor.transpose(
                    out=tr[:, even_off[i]: even_off[i] + C],
                    in_=feats[i][:, wb * 128:(wb + 1) * 128],
                    identity=ident[0:C, 0:C],
                )
            fo = fop.tile([128, CTOT], dt.float32, tag="fo", name=f"fo{wb}")
            for i, ch in enumerate(CHAINS):
                C = ch["C"]
                nc.scalar.copy(
                    out=fo[:, ch["out"]: ch["out"] + C],
                    in_=tr[:, even_off[i]: even_off[i] + C],
                )
            nc.sync.dma_start(out=fout[wb * 128:(wb + 1) * 128, :], in_=fo)

    nc.compile()
    return nc


def _host_consts(emb, Ws, bs):
    e = emb.astype(_BF16)
    etab = np.zeros((128, 256), dtype=_BF16)
    for h in range(2):
        etab[:, h * 128: h * 128 + 64] = e[h * 128:(h + 1) * 128, :]
        etab[:, h * 128 + 64: h * 128 + 128] = e[h * 128:(h + 1) * 128, :]

    iota = np.zeros((128, 2), dtype=np.float32)
    iota[:, 0] = np.arange(128)
    iota[:, 1] = np.arange(128, 256)

    wall = np.zeros((128, WALL_COLS), dtype=_BF16)
    for ki, W in enumerate(Ws):
        Wb = W.astype(np.float32)
        for (boff, K, bdt) in WALL_BLOCKS[ki]:
            c = CS[ki]
            wall[0:64, boff:boff + c] = Wb[:, :, bdt].T.astype(_BF16)
            if K == 128:
                wall[64:128, boff:boff + c] = Wb[:, :, bdt + 1].T.astype(_BF16)

    bias = np.zeros((128, NCH), dtype=np.float32)
    for i, ch in enumerate(CHAINS):
        bsl = bs[ch["ki"]][ch["clo"]: ch["clo"] + ch["C"]]
        bias[0:ch["C"], i] = bsl
    return etab, iota, wall, bias


def _host_ids(char_ids_core):
    """Per-core id plane [1, NGROUP*NIDXG] bf16 (PAD_ID in pad slots)."""
    flat = np.full((NGROUP, NIDXG), PAD_ID, dtype=np.float32)
    n = np.arange(GW)
    t = np.arange(LMAX)
    pos = DOFF + S * n[:, None] + t[None, :]
    cid = char_ids_core.astype(np.float32).reshape(NGROUP, GW, LMAX)
    for g in range(NGROUP):
        flat[g, pos] = cid[g]
    return flat.reshape(1, -1).astype(_BF16)


def kernel(**inputs):
    import jax

    jax.devices()  # boot the axon PJRT backend
    from concourse.bass_utils import run_bass_kernel_spmd

    char_ids = np.asarray(inputs["char_ids"], dtype=np.int32)
    word_pos = np.asarray(inputs["word_pos"], dtype=np.int64)
    word_batch = np.asarray(inputs["word_batch"], dtype=np.int64)
    emb = np.asarray(inputs["emb"], dtype=np.float32)
    Ws = [np.asarray(inputs[f"W{i+1}"], dtype=np.float32) for i in range(6)]
    bs = [np.asarray(inputs[f"b{i+1}"], dtype=np.float32) for i in range(6)]

    if "nc" not in _CACHE:
        _CACHE["nc"] = _build_program()
    nc = _CACHE["nc"]

    etab, iota, wall, bias = _host_consts(emb, Ws, bs)
    in_maps = []
    for c in range(NCORES):
        in_maps.append({
            "ids": _host_ids(char_ids[c * NWC:(c + 1) * NWC]),
            "etab": etab,
            "iota": iota,
            "wall": wall,
            "bias": bias,
        })

    core_ids = list(range(NCORES))
    trace = bool(os.environ.get("KERNEL_TRACE"))
    res = run_bass_kernel_spmd(nc, in_maps, core_ids, trace=trace)
    if trace:
        _CACHE["last_exec_time_ns"] = res.exec_time_ns

    f_full = np.concatenate([res.results[c]["f"] for c in core_ids], axis=0)

    out = np.zeros((WORDS, B, CTOT), dtype=np.float32)
    out[word_pos, word_batch] = f_full
    return out


# revision 13
# speedup vs baseline: 1.8864x; 1.0654x over previous
"""Trainium2 Bass kernel for nn_CNNEmbedding: char-CNN word embedding.

Reference computation (per flattened word, NW=16384 words):
  x = emb[char_ids]                       # [16, 64]
  for w in 1..6: y_w = conv1d(x.T, W_w, 'wide' pad) ; f_w = max_t tanh(y_w + b_w)
  f = concat(f_w)                         # [525]
  out[word_pos, word_batch] = f           # [256, 64, 525]

Kernel strategy (8 NeuronCores, data-parallel over words, 2048 words/core):
  - tanh is monotonic => max-pool BEFORE bias+tanh.
  - embedding lookup via one-hot matmul: ids broadcast to 128 partitions,
    VectorE tensor_scalar(is_equal) against a per-partition iota builds the
    one-hot [vocab-half, cols]; two accumulating matmuls against the
    (d-duplicated) embedding table produce x directly as
    [128 partitions (d dup), word-cols] in PSUM. Word blocks are strided
    21 cols (16 chars + 5 shared zero pad); pad slots use id 256 which
    matches no vocab row -> zero embedding.
  - ScalarE copies PSUM->SBUF: top half as-is, bottom half shifted left one
    column, so a single K=128 conv matmul computes TWO taps (dt, dt+1).
  - each conv = ceil(w/2) shifted matmuls accumulating in PSUM (fp32).
  - max over time: light chains via VectorE reduce_max from PSUM; heavy
    chains (k5, k6) via ScalarE PSUM->SBUF bf16 copy + GpSimd pairwise-max
    tree (engine load balancing).
  - ScalarE fused bias+tanh; TensorE transposes [C, words] -> [words, C]
    for contiguous output DMA.
"""

import os
import numpy as np
import ml_dtypes

# ---- problem constants (hardcoded; kernel.py must be self-contained) ----
B = 64
WORDS = 256
NW = B * WORDS          # 16384
LMAX = 16
V = 256
D = 64
KS = [1, 2, 3, 4, 5, 6]
CS = [25, 50, 75, 100, 125, 150]
CTOT = sum(CS)          # 525

NCORES = 8
NWC = NW // NCORES      # 2048 words per core
GW = 512                # words per group
NGROUP = NWC // GW      # 4
S = 21                  # word stride in x-plane (16 chars + 5 shared zero pad)
DOFF = 5                # first char col within a word block
NGC = S * GW + DOFF     # 10757 meaningful cols per group
NIDXG = ((NGC + 127) // 128) * 128   # 10880 cols per group plane
ECH = 512               # embed matmul chunk (one PSUM bank)
PAD_ID = 256            # pad id: matches no vocab row -> zero embedding

OUT_OFF = np.concatenate([[0], np.cumsum(CS)]).tolist()

_BF16 = ml_dtypes.bfloat16

_CACHE = {}


def _chains():
    ch = []
    for ki, (w, c) in enumerate(zip(KS, CS)):
        if c <= 128:
            ch.append(dict(ki=ki, w=w, C=c, clo=0, out=OUT_OFF[ki]))
        else:
            h = c // 2
            ch.append(dict(ki=ki, w=w, C=h, clo=0, out=OUT_OFF[ki]))
            ch.append(dict(ki=ki, w=w, C=c - h, clo=h, out=OUT_OFF[ki] + h))
    return ch


CHAINS = _chains()            # 7 chains
NCH = len(CHAINS)
# chains whose max-pool goes via ScalarE copy + GpSimd tree
# (disabled: trn2 walrus rejects max on the Pool engine)
OFFLOAD = set()

# wall (weight) block layout: per conv k, ceil(w/2) blocks of C_k columns.
WALL_BLOCKS = {}
_off = 0
for _ki, (_w, _c) in enumerate(zip(KS, CS)):
    blks = []
    for _p in range(_w // 2):
        blks.append((_off, 128, 2 * _p))
        _off += _c
    if _w % 2 == 1:
        blks.append((_off, 64, _w - 1))
        _off += _c
    WALL_BLOCKS[_ki] = blks
WALL_COLS = _off              # 1250


def _conv_items(mybir, nc, xp, wall_sb, psp, feats, scrp, g):
    """Yield emission callables for one group's conv work, per tile-group."""
    items = []
    for ci, ch in enumerate(CHAINS):
        w, C, clo = ch["w"], ch["C"], ch["clo"]
        T = LMAX + w - 1
        wpb = 512 // T
        blocks = WALL_BLOCKS[ch["ki"]]
        chunks = []
        n0 = 0
        while n0 < GW:
            chunks.append((n0, min(wpb, GW - n0)))
            n0 += wpb
        for t0 in range(0, len(chunks), 2):
            tg = chunks[t0:t0 + 2]
            items.append((ci, ch, T, blocks, tg))

    # round-robin across chains, heaviest chains first within each round
    by_chain = {}
    for it in items:
        by_chain.setdefault(it[0], []).append(it)
    order = sorted(by_chain, key=lambda c: -CHAINS[c]["w"])
    rr = []
    pos = 0
    while True:
        emitted = False
        for c in order:
            lst = by_chain[c]
            if pos < len(lst):
                rr.append(lst[pos])
                emitted = True
        if not emitted:
            break
        pos += 1
    return rr


def _build_program():
    from contextlib import ExitStack

    import concourse.mybir as mybir
    import concourse.tile as tile
    from concourse import bacc
    from concourse.masks import make_identity

    dt = mybir.dt
    nc = bacc.Bacc("TRN2", target_bir_lowering=False, debug=False,
                   num_devices=NCORES)

    idsd = nc.dram_tensor("ids", [1, NGROUP * NIDXG], dt.bfloat16,
                          kind="ExternalInput").ap()
    etab = nc.dram_tensor("etab", [128, 256], dt.bfloat16,
                          kind="ExternalInput").ap()
    iotad = nc.dram_tensor("iota", [128, 2], dt.float32,
                           kind="ExternalInput").ap()
    wall = nc.dram_tensor("wall", [128, WALL_COLS], dt.bfloat16,
                          kind="ExternalInput").ap()
    biasd = nc.dram_tensor("bias", [128, NCH], dt.float32,
                           kind="ExternalInput").ap()
    fout = nc.dram_tensor("f", [NWC, CTOT], dt.float32,
                          kind="ExternalOutput").ap()

    import concourse.bass as bass

    with tile.TileContext(nc) as tc, ExitStack() as ctx:
        singles = ctx.enter_context(tc.tile_pool(name="singles", bufs=1))
        idsp = ctx.enter_context(tc.tile_pool(name="idsp", bufs=2))
        ohp = ctx.enter_context(tc.tile_pool(name="ohp", bufs=4))
        xpp = ctx.enter_context(tc.tile_pool(name="xpp", bufs=2))
        psp = ctx.enter_context(tc.tile_pool(name="psp", bufs=3, space="PSUM"))
        psep = ctx.enter_context(tc.tile_pool(name="psep", bufs=1,
                                              space="PSUM"))
        trp = ctx.enter_context(tc.tile_pool(name="trp", bufs=1, space="PSUM"))
        scrp = ctx.enter_context(tc.tile_pool(name="scrp", bufs=4))
        fop = ctx.enter_context(tc.tile_pool(name="fop", bufs=2))

        etab_sb = singles.tile([128, 256], dt.bfloat16, tag="etab")
        nc.sync.dma_start(out=etab_sb, in_=etab)
        iota_sb = singles.tile([128, 2], dt.float32, tag="iota")
        nc.sync.dma_start(out=iota_sb, in_=iotad)
        wall_sb = singles.tile([128, WALL_COLS], dt.bfloat16, tag="wall")
        nc.sync.dma_start(out=wall_sb, in_=wall)
        bias_sb = singles.tile([128, NCH], dt.float32, tag="bias")
        nc.sync.dma_start(out=bias_sb, in_=biasd)
        ident = singles.tile([128, 128], dt.bfloat16, tag="ident")
        make_identity(nc, ident)

        feats = [
            singles.tile([ch["C"], NWC], dt.bfloat16, tag=f"feats{i}",
                         name=f"feats{i}")
            for i, ch in enumerate(CHAINS)
        ]

        def emit_embed_group(g):
            """Build xp plane for group g via one-hot matmuls. Returns a list
            of emission thunks (one per embed chunk-pair) so callers can weave
            them between conv items."""
            ids_sb = idsp.tile([128, NIDXG], dt.bfloat16, tag="ids",
                               name=f"ids{g}")
            bcast = bass.AP(
                tensor=idsd.tensor,
                offset=g * NIDXG,
                ap=[[0, 128], [1, NIDXG]],
            )
            nc.gpsimd.dma_start(out=ids_sb, in_=bcast)
            xp = xpp.tile([128, NIDXG], dt.bfloat16, tag="xp", name=f"xp{g}")

            thunks = []
            nch = (NIDXG + 2 * ECH - 1) // (2 * ECH)   # psum tiles of 2 chunks

            def mk(tci):
                def emit():
                    c0 = tci * 2 * ECH
                    cw = min(2 * ECH, NIDXG - c0)
                    # one-hot compares for both halves, full double-chunk
                    ohs = []
                    for h in range(2):
                        oh = ohp.tile([128, 2 * ECH], dt.bfloat16, tag="oh",
                                      name=f"oh{g}_{tci}_{h}")
                        nc.vector.tensor_scalar(
                            out=oh[:, 0:cw],
                            in0=ids_sb[:, c0: c0 + cw],
                            scalar1=iota_sb[:, h:h + 1],
                            scalar2=None,
                            op0=mybir.AluOpType.is_equal,
                        )
                        ohs.append(oh)
                    for j in range(0, cw, ECH):
                        n = min(ECH, cw - j)
                        pse = psep.tile([128, ECH], dt.float32, tag="pse",
                                        name=f"pse{g}_{tci}_{j}")
                        for h in range(2):
                            nc.tensor.matmul(
                                pse[:, 0:n],
                                lhsT=etab_sb[:, h * 128:(h + 1) * 128],
                                rhs=ohs[h][:, j:j + n],
                                start=(h == 0),
                                stop=(h == 1),
                            )
                        # copy to xp: top as-is, bottom shifted left one col
                        cj = c0 + j
                        nc.scalar.copy(out=xp[0:64, cj:cj + n],
                                       in_=pse[0:64, 0:n])
                        if cj == 0:
                            nc.scalar.copy(out=xp[64:128, 0:n - 1],
                                           in_=pse[64:128, 1:n])
                        else:
                            nc.scalar.copy(out=xp[64:128, cj - 1:cj + n - 1],
                                           in_=pse[64:128, 0:n])
                return emit

            for tci in range(nch):
                thunks.append(mk(tci))
            return xp, thunks

        def emit_conv_item(xp, g, item):
            ci, ch, T, blocks, tg = item
            C, clo, w = ch["C"], ch["clo"], ch["w"]
            ps = psp.tile([C, 2, 512], dt.float32, tag="ps",
                          name=f"ps{g}_{ci}_{tg[0][0]}")
            for j, (cn0, cnw) in enumerate(tg):
                for bi, (boff, K, bdt) in enumerate(blocks):
                    o = DOFF + bdt - (w - 1)
                    rhs = (
                        xp[0:K, S * cn0 + o: S * cn0 + o + S * cnw]
                        .rearrange("p (n t) -> p n t", t=S)[:, :, 0:T]
                    )
                    nc.tensor.matmul(
                        ps[:, j, 0:cnw * T],
                        lhsT=wall_sb[0:K, boff + clo: boff + clo + C],
                        rhs=rhs,
                        start=(bi == 0),
                        stop=(bi == len(blocks) - 1),
                    )
            if ci not in OFFLOAD:
                # VectorE grouped reduce straight out of PSUM
                r0 = 0
                while r0 < len(tg):
                    r1 = r0
                    while r1 < len(tg) and tg[r1][1] == tg[r0][1]:
                        r1 += 1
                    na, nwd = r1 - r0, tg[r0][1]
                    src = ps[:, r0:r1, 0:nwd * T].rearrange(
                        "c a (n t) -> c a n t", t=T)
                    w0 = g * GW + tg[r0][0]
                    dst = feats[ci][:, w0: w0 + na * nwd].rearrange(
                        "c (a n) -> c a n", n=nwd)
                    nc.vector.reduce_max(out=dst, in_=src,
                                         axis=mybir.AxisListType.X)
                    r0 = r1
            else:
                # ScalarE copy to SBUF bf16, then GpSimd pairwise-max tree
                na = len(tg)
                sca = scrp.tile([C, 2, 512], dt.bfloat16, tag="scra",
                                name=f"scra{g}_{ci}_{tg[0][0]}")
                scb = scrp.tile([C, 2, 512], dt.bfloat16, tag="scrb",
                                name=f"scrb{g}_{ci}_{tg[0][0]}")
                rc = 0
                while rc < len(tg):
                    rc1 = rc
                    while rc1 < len(tg) and tg[rc1][1] == tg[rc][1]:
                        rc1 += 1
                    nwd = tg[rc][1]
                    nc.scalar.copy(out=sca[:, rc:rc1, 0:nwd * T],
                                   in_=ps[:, rc:rc1, 0:nwd * T])
                    rc = rc1
                r0 = 0
                while r0 < len(tg):
                    r1 = r0
                    while r1 < len(tg) and tg[r1][1] == tg[r0][1]:
                        r1 += 1
                    nrun, nwd = r1 - r0, tg[r0][1]
                    w0 = g * GW + tg[r0][0]

                    def view(t4, Lc):
                        return t4[:, r0:r1, 0:nwd * T].rearrange(
                            "c a (n t) -> c a n t", t=T)[:, :, :, 0:Lc]

                    L = T
                    cur, nxt = sca, scb
                    while (L + 1) // 2 > 1:
                        Lh = (L + 1) // 2
                        a = view(cur, L)
                        nc.gpsimd.tensor_tensor(
                            out=view(nxt, Lh),
                            in0=a[:, :, :, 0:Lh],
                            in1=a[:, :, :, L - Lh:L],
                            op=mybir.AluOpType.max,
                        )
                        cur, nxt = nxt, cur
                        L = Lh
                    # final level (L == 2) writes feats directly
                    a = view(cur, L)
                    dstf = feats[ci][:, w0: w0 + nrun * nwd].rearrange(
                        "c (a n) -> c a n", n=nwd)
                    nc.gpsimd.tensor_tensor(
                        out=dstf,
                        in0=a[:, :, :, 0],
                        in1=a[:, :, :, L - 1],
                        op=mybir.AluOpType.max,
                    )
                    r0 = r1

        # prologue: embed group 0
        xp_cur, thunks = emit_embed_group(0)
        for t in thunks:
            t()

        for g in range(NGROUP):
            items = _conv_items(mybir, nc, xp_cur, wall_sb, psp, feats, scrp,
                                g)
            if g + 1 < NGROUP:
                xp_next, nthunks = emit_embed_group(g + 1)
            else:
                xp_next, nthunks = None, []
            # weave: distribute embed thunks of next group between conv items
            ne, ni = len(nthunks), len(items)
            ti = 0
            for k, item in enumerate(items):
                emit_conv_item(xp_cur, g, item)
                want = (k + 1) * ne // ni
                while ti < want:
                    nthunks[ti]()
                    ti += 1
            while ti < ne:
                nthunks[ti]()
                ti += 1
            xp_cur = xp_next

        # fused bias + tanh (in place, per chain)
        for i, ch in enumerate(CHAINS):
            nc.scalar.activation(
                out=feats[i], in_=feats[i],
                func=mybir.ActivationFunctionType.Tanh,
                bias=bias_sb[0:ch["C"], i:i + 1],
            )

        # transpose [C, words] -> [words, C] and DMA out. bf16 PSUM writes
        # need 4-byte alignment -> even column offsets in the staging tile.
        even_off = []
        _eo = 0
        for ch in CHAINS:
            even_off.append(_eo)
            _eo += ch["C"] + (ch["C"] % 2)
        for wb in range(NWC // 128):
            tr = trp.tile([128, _eo], dt.bfloat16, tag="tr",
                          name=f"tr{wb}")
            for i, ch in enumerate(CHAINS):
                C = ch["C"]
                nc.tensor.transpose(
                    out=tr[:, even_off[i]: even_off[i] + C],
                    in_=feats[i][:, wb * 128:(wb + 1) * 128],
                    identity=ident[0:C, 0:C],
                )
            fo = fop.tile([128, CTOT], dt.float32, tag="fo", name=f"fo{wb}")
            for i, ch in enumerate(CHAINS):
                C = ch["C"]
                nc.scalar.copy(
                    out=fo[:, ch["out"]: ch["out"] + C],
                    in_=tr[:, even_off[i]: even_off[i] + C],
                )
            nc.sync.dma_start(out=fout[wb * 128:(wb + 1) * 128, :], in_=fo)

    nc.compile()
    return nc


def _host_consts(emb, Ws, bs):
    e = emb.astype(_BF16)
    etab = np.zeros((128, 256), dtype=_BF16)
    for h in range(2):
        etab[:, h * 128: h * 128 + 64] = e[h * 128:(h + 1) * 128, :]
        etab[:, h * 128 + 64: h * 128 + 128] = e[h * 128:(h + 1) * 128, :]

    iota = np.zeros((128, 2), dtype=np.float32)
    iota[:, 0] = np.arange(128)
    iota[:, 1] = np.arange(128, 256)

    wall = np.zeros((128, WALL_COLS), dtype=_BF16)
    for ki, W in enumerate(Ws):
        Wb = W.astype(np.float32)
        for (boff, K, bdt) in WALL_BLOCKS[ki]:
            c = CS[ki]
            wall[0:64, boff:boff + c] = Wb[:, :, bdt].T.astype(_BF16)
            if K == 128:
                wall[64:128, boff:boff + c] = Wb[:, :, bdt + 1].T.astype(_BF16)

    bias = np.zeros((128, NCH), dtype=np.float32)
    for i, ch in enumerate(CHAINS):
        bsl = bs[ch["ki"]][ch["clo"]: ch["clo"] + ch["C"]]
        bias[0:ch["C"], i] = bsl
    return etab, iota, wall, bias


def _host_ids(char_ids_core):
    """Per-core id plane [1, NGROUP*NIDXG] bf16 (PAD_ID in pad slots)."""
    flat = np.full((NGROUP, NIDXG), PAD_ID, dtype=np.float32)
    n = np.arange(GW)
    t = np.arange(LMAX)
    pos = DOFF + S * n[:, None] + t[None, :]
    cid = char_ids_core.astype(np.float32).reshape(NGROUP, GW, LMAX)
    for g in range(NGROUP):
        flat[g, pos] = cid[g]
    return flat.reshape(1, -1).astype(_BF16)


def kernel(**inputs):
    import jax

    jax.devices()  # boot the axon PJRT backend
    from concourse.bass_utils import run_bass_kernel_spmd

    char_ids = np.asarray(inputs["char_ids"], dtype=np.int32)
    word_pos = np.asarray(inputs["word_pos"], dtype=np.int64)
    word_batch = np.asarray(inputs["word_batch"], dtype=np.int64)
    emb = np.asarray(inputs["emb"], dtype=np.float32)
    Ws = [np.asarray(inputs[f"W{i+1}"], dtype=np.float32) for i in range(6)]
    bs = [np.asarray(inputs[f"b{i+1}"], dtype=np.float32) for i in range(6)]

    if "nc" not in _CACHE:
        _CACHE["nc"] = _build_program()
    nc = _CACHE["nc"]

    etab, iota, wall, bias = _host_consts(emb, Ws, bs)
    in_maps = []
    for c in range(NCORES):
        in_maps.append({
            "ids": _host_ids(char_ids[c * NWC:(c + 1) * NWC]),
            "etab": etab,
            "iota": iota,
            "wall": wall,
            "bias": bias,
        })

    core_ids = list(range(NCORES))
    trace = bool(os.environ.get("KERNEL_TRACE"))
    res = run_bass_kernel_spmd(nc, in_maps, core_ids, trace=trace)
    if trace:
        _CACHE["last_exec_time_ns"] = res.exec_time_ns

    f_full = np.concatenate([res.results[c]["f"] for c in core_ids], axis=0)

    out = np.zeros((WORDS, B, CTOT), dtype=np.float32)
    out[word_pos, word_batch] = f_full
    return out


# revision 14
# speedup vs baseline: 2.0327x; 1.0775x over previous
"""Trainium2 Bass kernel for nn_CNNEmbedding: char-CNN word embedding.

Reference computation (per flattened word, NW=16384 words):
  x = emb[char_ids]                       # [16, 64]
  for w in 1..6: y_w = conv1d(x.T, W_w, 'wide' pad) ; f_w = max_t tanh(y_w + b_w)
  f = concat(f_w)                         # [525]
  out[word_pos, word_batch] = f           # [256, 64, 525]

Kernel strategy (8 NeuronCores, data-parallel over words, 2048 words/core):
  - tanh is monotonic => max-pool BEFORE bias+tanh.
  - embedding lookup via one-hot matmul: ids broadcast to 128 partitions,
    VectorE tensor_scalar(is_equal) against a per-partition iota builds the
    one-hot [vocab-half, cols]; two accumulating matmuls against the
    (d-duplicated) embedding table produce x directly as
    [128 partitions (d dup), word-cols] in PSUM. Word blocks are strided
    21 cols (16 chars + 5 shared zero pad); pad slots use id 256 which
    matches no vocab row -> zero embedding.
  - ScalarE copies PSUM->SBUF: top half as-is, bottom half shifted left one
    column, so a single K=128 conv matmul computes TWO taps (dt, dt+1).
  - each conv = ceil(w/2) shifted matmuls accumulating in PSUM (fp32).
  - max over time: light chains via VectorE reduce_max from PSUM; heavy
    chains (k5, k6) via ScalarE PSUM->SBUF bf16 copy + GpSimd pairwise-max
    tree (engine load balancing).
  - ScalarE fused bias+tanh; TensorE transposes [C, words] -> [words, C]
    for contiguous output DMA.
"""

import os
import numpy as np
import ml_dtypes

# ---- problem constants (hardcoded; kernel.py must be self-contained) ----
B = 64
WORDS = 256
NW = B * WORDS          # 16384
LMAX = 16
V = 256
D = 64
KS = [1, 2, 3, 4, 5, 6]
CS = [25, 50, 75, 100, 125, 150]
CTOT = sum(CS)          # 525

NCORES = 8
NWC = NW // NCORES      # 2048 words per core
GW = 512                # words per group
NGROUP = NWC // GW      # 4
S = 21                  # word stride in x-plane (16 chars + 5 shared zero pad)
DOFF = 5                # first char col within a word block
NGC = S * GW + DOFF     # 10757 meaningful cols per group
NIDXG = ((NGC + 127) // 128) * 128   # 10880 cols per group plane
ECH = 512               # embed matmul chunk (one PSUM bank)
PAD_ID = 256            # pad id: matches no vocab row -> zero embedding

OUT_OFF = np.concatenate([[0], np.cumsum(CS)]).tolist()

_BF16 = ml_dtypes.bfloat16

_CACHE = {}


def _chains():
    ch = []
    for ki, (w, c) in enumerate(zip(KS, CS)):
        if c <= 128:
            ch.append(dict(ki=ki, w=w, C=c, clo=0, out=OUT_OFF[ki]))
        else:
            h = c // 2
            ch.append(dict(ki=ki, w=w, C=h, clo=0, out=OUT_OFF[ki]))
            ch.append(dict(ki=ki, w=w, C=c - h, clo=h, out=OUT_OFF[ki] + h))
    return ch


CHAINS = _chains()            # 7 chains
NCH = len(CHAINS)
# chains whose max-pool goes via ScalarE copy + GpSimd tree
# (disabled: trn2 walrus rejects max on the Pool engine)
OFFLOAD = set()

# wall (weight) block layout: per conv k, ceil(w/2) blocks of C_k columns.
WALL_BLOCKS = {}
_off = 0
for _ki, (_w, _c) in enumerate(zip(KS, CS)):
    blks = []
    for _p in range(_w // 2):
        blks.append((_off, 128, 2 * _p))
        _off += _c
    if _w % 2 == 1:
        blks.append((_off, 64, _w - 1))
        _off += _c
    WALL_BLOCKS[_ki] = blks
WALL_COLS = _off              # 1250


def _conv_items(mybir, nc, xp, wall_sb, psp, feats, scrp, g):
    """Yield emission callables for one group's conv work, per tile-group."""
    items = []
    for ci, ch in enumerate(CHAINS):
        w, C, clo = ch["w"], ch["C"], ch["clo"]
        T = LMAX + w - 1
        wpb = 512 // T
        blocks = WALL_BLOCKS[ch["ki"]]
        chunks = []
        n0 = 0
        while n0 < GW:
            chunks.append((n0, min(wpb, GW - n0)))
            n0 += wpb
        for t0 in range(0, len(chunks), 2):
            tg = chunks[t0:t0 + 2]
            items.append((ci, ch, T, blocks, tg))

    # round-robin across chains, heaviest chains first within each round
    by_chain = {}
    for it in items:
        by_chain.setdefault(it[0], []).append(it)
    order = sorted(by_chain, key=lambda c: -CHAINS[c]["w"])
    rr = []
    pos = 0
    while True:
        emitted = False
        for c in order:
            lst = by_chain[c]
            if pos < len(lst):
                rr.append(lst[pos])
                emitted = True
        if not emitted:
            break
        pos += 1
    return rr


def _build_program():
    from contextlib import ExitStack

    import concourse.mybir as mybir
    import concourse.tile as tile
    from concourse import bacc
    from concourse.masks import make_identity

    dt = mybir.dt
    nc = bacc.Bacc("TRN2", target_bir_lowering=False, debug=False,
                   num_devices=NCORES)

    idsd = nc.dram_tensor("ids", [1, NGROUP * NIDXG], dt.bfloat16,
                          kind="ExternalInput").ap()
    etab = nc.dram_tensor("etab", [128, 256], dt.bfloat16,
                          kind="ExternalInput").ap()
    iotad = nc.dram_tensor("iota", [128, 2], dt.float32,
                           kind="ExternalInput").ap()
    wall = nc.dram_tensor("wall", [128, WALL_COLS], dt.bfloat16,
                          kind="ExternalInput").ap()
    biasd = nc.dram_tensor("bias", [128, NCH], dt.float32,
                           kind="ExternalInput").ap()
    fout = nc.dram_tensor("f", [NWC, CTOT], dt.float32,
                          kind="ExternalOutput").ap()

    import concourse.bass as bass

    with tile.TileContext(nc) as tc, ExitStack() as ctx:
        singles = ctx.enter_context(tc.tile_pool(name="singles", bufs=1))
        idsp = ctx.enter_context(tc.tile_pool(name="idsp", bufs=2))
        ohp = ctx.enter_context(tc.tile_pool(name="ohp", bufs=4))
        xpp = ctx.enter_context(tc.tile_pool(name="xpp", bufs=2))
        psp = ctx.enter_context(tc.tile_pool(name="psp", bufs=3, space="PSUM"))
        psep = ctx.enter_context(tc.tile_pool(name="psep", bufs=1,
                                              space="PSUM"))
        trp = ctx.enter_context(tc.tile_pool(name="trp", bufs=1, space="PSUM"))
        scrp = ctx.enter_context(tc.tile_pool(name="scrp", bufs=4))
        fop = ctx.enter_context(tc.tile_pool(name="fop", bufs=2))

        etab_sb = singles.tile([128, 256], dt.bfloat16, tag="etab")
        nc.sync.dma_start(out=etab_sb, in_=etab)
        iota_sb = singles.tile([128, 2], dt.float32, tag="iota")
        nc.sync.dma_start(out=iota_sb, in_=iotad)
        wall_sb = singles.tile([128, WALL_COLS], dt.bfloat16, tag="wall")
        nc.sync.dma_start(out=wall_sb, in_=wall)
        bias_sb = singles.tile([128, NCH], dt.float32, tag="bias")
        nc.sync.dma_start(out=bias_sb, in_=biasd)
        ident = singles.tile([128, 128], dt.bfloat16, tag="ident")
        make_identity(nc, ident)

        feats = [
            singles.tile([ch["C"], NWC], dt.bfloat16, tag=f"feats{i}",
                         name=f"feats{i}")
            for i, ch in enumerate(CHAINS)
        ]

        def emit_embed_group(g):
            """Build xp plane for group g via one-hot matmuls. Returns a list
            of emission thunks (one per embed chunk-pair) so callers can weave
            them between conv items."""
            ids_sb = idsp.tile([128, NIDXG], dt.bfloat16, tag="ids",
                               name=f"ids{g}")
            bcast = bass.AP(
                tensor=idsd.tensor,
                offset=g * NIDXG,
                ap=[[0, 128], [1, NIDXG]],
            )
            nc.gpsimd.dma_start(out=ids_sb, in_=bcast)
            xp = xpp.tile([128, NIDXG], dt.bfloat16, tag="xp", name=f"xp{g}")

            thunks = []
            nch = (NIDXG + 2 * ECH - 1) // (2 * ECH)   # psum tiles of 2 chunks

            def mk(tci):
                def emit():
                    c0 = tci * 2 * ECH
                    cw = min(2 * ECH, NIDXG - c0)
                    # one-hot compares for both halves, full double-chunk
                    ohs = []
                    for h in range(2):
                        oh = ohp.tile([128, 2 * ECH], dt.bfloat16, tag="oh",
                                      name=f"oh{g}_{tci}_{h}")
                        nc.vector.tensor_scalar(
                            out=oh[:, 0:cw],
                            in0=ids_sb[:, c0: c0 + cw],
                            scalar1=iota_sb[:, h:h + 1],
                            scalar2=None,
                            op0=mybir.AluOpType.is_equal,
                        )
                        ohs.append(oh)
                    for j in range(0, cw, ECH):
                        n = min(ECH, cw - j)
                        pse = psep.tile([128, ECH], dt.float32, tag="pse",
                                        name=f"pse{g}_{tci}_{j}")
                        for h in range(2):
                            nc.tensor.matmul(
                                pse[:, 0:n],
                                lhsT=etab_sb[:, h * 128:(h + 1) * 128],
                                rhs=ohs[h][:, j:j + n],
                                start=(h == 0),
                                stop=(h == 1),
                            )
                        # copy to xp: top as-is, bottom shifted left one col
                        cj = c0 + j
                        nc.scalar.copy(out=xp[0:64, cj:cj + n],
                                       in_=pse[0:64, 0:n])
                        if cj == 0:
                            nc.scalar.copy(out=xp[64:128, 0:n - 1],
                                           in_=pse[64:128, 1:n])
                        else:
                            nc.scalar.copy(out=xp[64:128, cj - 1:cj + n - 1],
                                           in_=pse[64:128, 0:n])
                return emit

            for tci in range(nch):
                thunks.append(mk(tci))
            return xp, thunks

        def emit_conv_item(xp, g, item):
            ci, ch, T, blocks, tg = item
            C, clo, w = ch["C"], ch["clo"], ch["w"]
            ps = psp.tile([C, 2, 512], dt.float32, tag="ps",
                          name=f"ps{g}_{ci}_{tg[0][0]}")
            for j, (cn0, cnw) in enumerate(tg):
                for bi, (boff, K, bdt) in enumerate(blocks):
                    o = DOFF + bdt - (w - 1)
                    rhs = (
                        xp[0:K, S * cn0 + o: S * cn0 + o + S * cnw]
                        .rearrange("p (n t) -> p n t", t=S)[:, :, 0:T]
                    )
                    nc.tensor.matmul(
                        ps[:, j, 0:cnw * T],
                        lhsT=wall_sb[0:K, boff + clo: boff + clo + C],
                        rhs=rhs,
                        start=(bi == 0),
                        stop=(bi == len(blocks) - 1),
                    )
            if ci not in OFFLOAD:
                # VectorE grouped reduce straight out of PSUM
                r0 = 0
                while r0 < len(tg):
                    r1 = r0
                    while r1 < len(tg) and tg[r1][1] == tg[r0][1]:
                        r1 += 1
                    na, nwd = r1 - r0, tg[r0][1]
                    src = ps[:, r0:r1, 0:nwd * T].rearrange(
                        "c a (n t) -> c a n t", t=T)
                    w0 = g * GW + tg[r0][0]
                    dst = feats[ci][:, w0: w0 + na * nwd].rearrange(
                        "c (a n) -> c a n", n=nwd)
                    nc.vector.reduce_max(out=dst, in_=src,
                                         axis=mybir.AxisListType.X)
                    r0 = r1
            else:
                # ScalarE copy to SBUF bf16, then GpSimd pairwise-max tree
                na = len(tg)
                sca = scrp.tile([C, 2, 512], dt.bfloat16, tag="scra",
                                name=f"scra{g}_{ci}_{tg[0][0]}")
                scb = scrp.tile([C, 2, 512], dt.bfloat16, tag="scrb",
                                name=f"scrb{g}_{ci}_{tg[0][0]}")
                rc = 0
                while rc < len(tg):
                    rc1 = rc
                    while rc1 < len(tg) and tg[rc1][1] == tg[rc][1]:
                        rc1 += 1
                    nwd = tg[rc][1]
                    nc.scalar.copy(out=sca[:, rc:rc1, 0:nwd * T],
                                   in_=ps[:, rc:rc1, 0:nwd * T])
                    rc = rc1
                r0 = 0
                while r0 < len(tg):
                    r1 = r0
                    while r1 < len(tg) and tg[r1][1] == tg[r0][1]:
                        r1 += 1
                    nrun, nwd = r1 - r0, tg[r0][1]
                    w0 = g * GW + tg[r0][0]

                    def view(t4, Lc):
                        return t4[:, r0:r1, 0:nwd * T].rearrange(
                            "c a (n t) -> c a n t", t=T)[:, :, :, 0:Lc]

                    L = T
                    cur, nxt = sca, scb
                    while (L + 1) // 2 > 1:
                        Lh = (L + 1) // 2
                        a = view(cur, L)
                        nc.gpsimd.tensor_tensor(
                            out=view(nxt, Lh),
                            in0=a[:, :, :, 0:Lh],
                            in1=a[:, :, :, L - Lh:L],
                            op=mybir.AluOpType.max,
                        )
                        cur, nxt = nxt, cur
                        L = Lh
                    # final level (L == 2) writes feats directly
                    a = view(cur, L)
                    dstf = feats[ci][:, w0: w0 + nrun * nwd].rearrange(
                        "c (a n) -> c a n", n=nwd)
                    nc.gpsimd.tensor_tensor(
                        out=dstf,
                        in0=a[:, :, :, 0],
                        in1=a[:, :, :, L - 1],
                        op=mybir.AluOpType.max,
                    )
                    r0 = r1

        # even column offsets for bf16 PSUM 4-byte alignment
        even_off = []
        _eo = 0
        for ch in CHAINS:
            even_off.append(_eo)
            _eo += ch["C"] + (ch["C"] % 2)

        def emit_output_group(g):
            """bias+tanh for group g's word range, then transpose + DMA out
            its four 128-word blocks (overlaps next group's convs)."""
            w0 = g * GW
            for i, ch in enumerate(CHAINS):
                nc.scalar.activation(
                    out=feats[i][:, w0:w0 + GW],
                    in_=feats[i][:, w0:w0 + GW],
                    func=mybir.ActivationFunctionType.Tanh,
                    bias=bias_sb[0:ch["C"], i:i + 1],
                )
            for wb in range(w0 // 128, (w0 + GW) // 128):
                tr = trp.tile([128, _eo], dt.bfloat16, tag="tr",
                              name=f"tr{wb}")
                for i, ch in enumerate(CHAINS):
                    C = ch["C"]
                    nc.tensor.transpose(
                        out=tr[:, even_off[i]: even_off[i] + C],
                        in_=feats[i][:, wb * 128:(wb + 1) * 128],
                        identity=ident[0:C, 0:C],
                    )
                fo = fop.tile([128, CTOT], dt.float32, tag="fo",
                              name=f"fo{wb}")
                for i, ch in enumerate(CHAINS):
                    C = ch["C"]
                    nc.scalar.copy(
                        out=fo[:, ch["out"]: ch["out"] + C],
                        in_=tr[:, even_off[i]: even_off[i] + C],
                    )
                nc.sync.dma_start(out=fout[wb * 128:(wb + 1) * 128, :],
                                  in_=fo)

        # prologue: embed group 0
        xp_cur, thunks = emit_embed_group(0)
        for t in thunks:
            t()

        for g in range(NGROUP):
            items = _conv_items(mybir, nc, xp_cur, wall_sb, psp, feats, scrp,
                                g)
            if g + 1 < NGROUP:
                xp_next, nthunks = emit_embed_group(g + 1)
            else:
                xp_next, nthunks = None, []
            # weave: distribute embed thunks of next group between conv items
            ne, ni = len(nthunks), len(items)
            ti = 0
            for k, item in enumerate(items):
                emit_conv_item(xp_cur, g, item)
                want = (k + 1) * ne // ni
                while ti < want:
                    nthunks[ti]()
                    ti += 1
            while ti < ne:
                nthunks[ti]()
                ti += 1
            if g > 0:
                emit_output_group(g - 1)
            xp_cur = xp_next
        emit_output_group(NGROUP - 1)

    nc.compile()
    return nc


def _host_consts(emb, Ws, bs):
    e = emb.astype(_BF16)
    etab = np.zeros((128, 256), dtype=_BF16)
    for h in range(2):
        etab[:, h * 128: h * 128 + 64] = e[h * 128:(h + 1) * 128, :]
        etab[:, h * 128 + 64: h * 128 + 128] = e[h * 128:(h + 1) * 128, :]

    iota = np.zeros((128, 2), dtype=np.float32)
    iota[:, 0] = np.arange(128)
    iota[:, 1] = np.arange(128, 256)

    wall = np.zeros((128, WALL_COLS), dtype=_BF16)
    for ki, W in enumerate(Ws):
        Wb = W.astype(np.float32)
        for (boff, K, bdt) in WALL_BLOCKS[ki]:
            c = CS[ki]
            wall[0:64, boff:boff + c] = Wb[:, :, bdt].T.astype(_BF16)
            if K == 128:
                wall[64:128, boff:boff + c] = Wb[:, :, bdt + 1].T.astype(_BF16)

    bias = np.zeros((128, NCH), dtype=np.float32)
    for i, ch in enumerate(CHAINS):
        bsl = bs[ch["ki"]][ch["clo"]: ch["clo"] + ch["C"]]
        bias[0:ch["C"], i] = bsl
    return etab, iota, wall, bias


def _host_ids(char_ids_core):
    """Per-core id plane [1, NGROUP*NIDXG] bf16 (PAD_ID in pad slots)."""
    flat = np.full((NGROUP, NIDXG), PAD_ID, dtype=np.float32)
    n = np.arange(GW)
    t = np.arange(LMAX)
    pos = DOFF + S * n[:, None] + t[None, :]
    cid = char_ids_core.astype(np.float32).reshape(NGROUP, GW, LMAX)
    for g in range(NGROUP):
        flat[g, pos] = cid[g]
    return flat.reshape(1, -1).astype(_BF16)


def kernel(**inputs):
    import jax

    jax.devices()  # boot the axon PJRT backend
    from concourse.bass_utils import run_bass_kernel_spmd

    char_ids = np.asarray(inputs["char_ids"], dtype=np.int32)
    word_pos = np.asarray(inputs["word_pos"], dtype=np.int64)
    word_batch = np.asarray(inputs["word_batch"], dtype=np.int64)
    emb = np.asarray(inputs["emb"], dtype=np.float32)
    Ws = [np.asarray(inputs[f"W{i+1}"], dtype=np.float32) for i in range(6)]
    bs = [np.asarray(inputs[f"b{i+1}"], dtype=np.float32) for i in range(6)]

    if "nc" not in _CACHE:
        _CACHE["nc"] = _build_program()
    nc = _CACHE["nc"]

    etab, iota, wall, bias = _host_consts(emb, Ws, bs)
    in_maps = []
    for c in range(NCORES):
        in_maps.append({
            "ids": _host_ids(char_ids[c * NWC:(c + 1) * NWC]),
            "etab": etab,
            "iota": iota,
            "wall": wall,
            "bias": bias,
        })

    core_ids = list(range(NCORES))
    trace = bool(os.environ.get("KERNEL_TRACE"))
    res = run_bass_kernel_spmd(nc, in_maps, core_ids, trace=trace)
    if trace:
        _CACHE["last_exec_time_ns"] = res.exec_time_ns

    f_full = np.concatenate([res.results[c]["f"] for c in core_ids], axis=0)

    out = np.zeros((WORDS, B, CTOT), dtype=np.float32)
    out[word_pos, word_batch] = f_full
    return out
